# revision 12
# baseline (speedup 1.0000x reference)
"""CPAB transformer kernel for Trainium2 (8 NeuronCores, SPMD).

Problem: 1D CPAB warp. points [1, 262144] f32, theta [8, 30], basis [64, 30].
reference:
    Avees = basis @ theta.T ; As = Avees.T.reshape(8*32, 1, 2)
    Trels = expm(dT*As) -> per (theta, cell): x' = A_c * x + B_c
    32 steps of: c = clip(floor(32 x), 0, 31); x = A_c x + B_c
    out[t, 0, n] = final x for theta t, point n.

Strategy: all 32 steps apply the SAME piecewise-affine map G, so the final
map is the 32-fold self-composition G^32 -- piecewise affine with ~1-2k
breakpoints.  G^32 depends only on (theta, basis), so it is composed exactly
on the host (float64; pure theta preprocessing -- all per-point work stays
on device) and approximated on a uniform grid of M = 8192 cells over [0,1)
in CELL-LOCAL coordinates:  y ~= TP[c]*u + TQ[c]  with c = floor(M x),
u = M x - c in [0,1).  Per-cell coefficients are the least-squares affine
fit over the cell (exact for the ~80% of cells without a breakpoint).  The
local parametrization keeps TP tiny, so fp16 tables cost no accuracy
(breakpoint cells have steep compensating fits that would cancel
catastrophically in global coordinates).  Measured end-to-end rel err
5.3e-3 (tolerance 2e-2, fp32 device baseline was 1.7e-3).

Device: points are sharded across the 8 NeuronCores (32768 each).  The grid
index c is THETA-INDEPENDENT, which exactly matches the gpsimd ap_gather
index model: the 16 partitions of each Q7 core share one index stream.  The
16 channels of each Q7 core hold the 8 per-theta (TP,TQ) fp16 pair-tables
(theta = channel % 8), so ONE ap_gather per chunk fetches the pair for all
8 thetas at once with zero redundancy (~28 ns per point covers all thetas).
The u tensor is recomputed on the replicated layout by the DVE (exact, the
same fp32 ops as the index path) and the fma runs on strided fp16 planes,
all overlapped under the next chunk's gather.

x_rep / xw are host-staged replications of the point slice so the engine
layouts line up; row 16*j + m of the output is theta (m % 8)'s result for
the Q7-core-j point stream (m >= 8 rows are duplicates, ignored on unshard).
"""

import numpy as np

NC = 32
NSTEPS = 32
N_THETA = 8
N_POINTS = 262144
P = 128

NCORES = 8           # NeuronCores (SPMD)
NPC = N_POINTS // NCORES   # points per NeuronCore = 32768
NI = NPC // 8        # ap_gather num_idxs per Q7 core = 4096
LOGM = 13
M = 1 << LOGM        # uniform grid cells on [0,1)  (M*d <= 32768: int16 offsets)
NCHUNK = 4           # gather chunks (pipeline fma/out under the gather)
NIC = NI // NCHUNK

_PROGRAM = None


# ----------------------------- host: compose G^32 -----------------------------

def _host_cell_tables(theta, basis):
    dT = 1.0 / NSTEPS
    Avees = basis.astype(np.float64) @ theta.astype(np.float64).T
    As = Avees.T.reshape(theta.shape[0] * NC, 2)
    a = dT * As[:, 0]
    b = dT * As[:, 1]
    small = np.abs(a) < 1e-6
    a_safe = np.where(small, 1.0, a)
    phi = np.where(small, 1.0 + 0.5 * a, np.expm1(a_safe) / a_safe)
    A = np.exp(a).reshape(theta.shape[0], NC)
    B = (b * phi).reshape(theta.shape[0], NC)
    return A, B


def _compose_map(A_t, B_t, lo=-4.0, hi=4.0):
    """Exact NSTEPS-fold self-composition of x -> A[c]x+B[c], c=clip(floor(NC x)).
    Returns (bps, Pc, Qc): segment i covers (edges[i], edges[i+1]) with
    y = Pc[i] x + Qc[i], edges = [lo] + bps + [hi]."""
    bps = np.empty(0)
    Pc = np.array([1.0])
    Qc = np.array([0.0])
    for _ in range(NSTEPS):
        edges = np.concatenate([[lo], bps, [hi]])
        l, r = edges[:-1], edges[1:]
        vl = Pc * l + Qc
        vr = Pc * r + Qc
        vmin = np.minimum(vl, vr)
        vmax = np.maximum(vl, vr)
        k0 = np.maximum(np.floor(vmin * NC).astype(np.int64) + 1, 1)
        k1 = np.minimum(np.ceil(vmax * NC).astype(np.int64) - 1, NC - 1)
        cnt = np.maximum(k1 - k0 + 1, 0)
        seg_ids = np.repeat(np.arange(len(Pc)), cnt)
        if len(seg_ids):
            ks = np.concatenate(
                [np.arange(a0, a0 + c) for a0, c in zip(k0, cnt) if c > 0]
            )
            xk = (ks / NC - Qc[seg_ids]) / Pc[seg_ids]
            inside = (xk > l[seg_ids]) & (xk < r[seg_ids])
            newbps = np.unique(np.concatenate([bps, xk[inside]]))
        else:
            newbps = bps
        edges = np.concatenate([[lo], newbps, [hi]])
        mids = (edges[:-1] + edges[1:]) / 2
        seg = np.searchsorted(bps, mids)
        fm = Pc[seg] * mids + Qc[seg]
        c = np.clip(np.floor(fm * NC), 0, NC - 1).astype(np.int64)
        Pc, Qc = A_t[c] * Pc[seg], A_t[c] * Qc[seg] + B_t[c]
        bps = newbps
    return bps, Pc, Qc


def _grid_tables(theta, basis):
    """[n_theta, M, 2] fp16 local-coordinate tables (y ~= TP*u + TQ), as the
    uint16 wire format.  Least-squares affine per cell (exact where the cell
    contains no breakpoint of the composed map)."""
    A, B = _host_cell_tables(theta, basis)
    n_theta = theta.shape[0]
    out = np.empty((n_theta, M, 2), dtype=np.float16)
    grid = np.arange(M) / M
    mids = grid + 0.5 / M
    for t in range(n_theta):
        bps, Pc, Qc = _compose_map(A[t], B[t])
        seg = np.searchsorted(bps, mids)
        TP = Pc[seg] / M
        TQ = Pc[seg] * grid + Qc[seg]
        inner = bps[(bps > 0) & (bps < 1)]
        for c in np.unique((inner * M).astype(np.int64)):
            lo0, hi0 = c / M, (c + 1) / M
            e = np.concatenate(
                [[lo0], inner[(inner > lo0) & (inner < hi0)], [hi0]]
            )
            m0 = m1 = m2 = mF = mxF = 0.0
            for i in range(len(e) - 1):
                lo, hi = e[i], e[i + 1]
                s = np.searchsorted(bps, (lo + hi) / 2)
                p, q = Pc[s], Qc[s]
                w = hi - lo
                i1 = (hi**2 - lo**2) / 2
                i2 = (hi**3 - lo**3) / 3
                m0 += w
                m1 += i1
                m2 += i2
                mF += p * i1 + q * w
                mxF += p * i2 + q * i1
            det = m2 * m0 - m1 * m1
            pstar = (mxF * m0 - mF * m1) / det
            qstar = (mF * m2 - mxF * m1) / det
            TP[c] = pstar / M
            TQ[c] = pstar * lo0 + qstar
        out[t, :, 0] = TP.astype(np.float16)
        out[t, :, 1] = TQ.astype(np.float16)
    return out.view(np.uint16)


# ----------------------------- device program ---------------------------------

def _build_program():
    global _PROGRAM
    if _PROGRAM is not None:
        return _PROGRAM
    import concourse.bacc as bacc
    import concourse.mybir as mybir
    from concourse import library_config
    from concourse.tile import TileContext

    f32 = mybir.dt.float32
    f16 = mybir.dt.float16
    i16 = mybir.dt.int16
    mult = mybir.AluOpType.mult
    add = mybir.AluOpType.add
    sub = mybir.AluOpType.subtract
    amin = mybir.AluOpType.min
    amax = mybir.AluOpType.max

    nc = bacc.Bacc("TRN2", target_bir_lowering=False, debug=False,
                   num_devices=NCORES)
    tabs = nc.dram_tensor("tabs", [P, M * 2], f16, kind="ExternalInput").ap()
    xw = nc.dram_tensor("xw", [P, NI // 16], f32, kind="ExternalInput").ap()
    xr = nc.dram_tensor("xr", [P, NI], f32, kind="ExternalInput").ap()
    out = nc.dram_tensor("out", [P, NI], f32, kind="ExternalOutput").ap()

    big = float(2**23)
    with TileContext(nc) as tc:
        with tc.tile_pool(name="state", bufs=1) as pool:
            tb = pool.tile([P, M * 2], f16, tag="tab")
            xwb = pool.tile([P, NI // 16], f32, tag="xw")
            wb = pool.tile([P, NI // 16], f32, tag="w")
            ibs = [
                pool.tile([P, NIC // 16], i16, tag=f"idx{ch}", name=f"ib{ch}")
                for ch in range(NCHUNK)
            ]
            xrb = pool.tile([P, NI], f32, tag="xr")
            vrb = pool.tile([P, NI], f32, tag="vr")
            wrb = pool.tile([P, NI], f32, tag="wr")
            pqs = [
                pool.tile([P, NIC * 2], f16, tag=f"pq{ch}", name=f"pq{ch}")
                for ch in range(NCHUNK)
            ]
            yb = pool.tile([P, NI], f32, tag="y")
            nc.gpsimd.dma_start(xwb[:], xw[:])
            nc.scalar.dma_start(tb[:], tabs[:])
            nc.sync.dma_start(xrb[:], xr[:])
            nc.gpsimd.load_library(library_config.ap_gather)
            # c = clip(floor(x * M), 0, M-1) via the 2^23 trick, exact in fp32
            nc.vector.tensor_scalar(wb[:], xwb[:], float(M), float(M) - 0.5, mult, amin)
            nc.vector.tensor_scalar(wb[:], wb[:], 0.0, big - 0.5, amax, add)
            for ch in range(NCHUNK):
                si = slice(ch * NIC // 16, (ch + 1) * NIC // 16)
                nc.vector.tensor_scalar(
                    ibs[ch][:], wb[:, si], big, 0.0, sub, add
                )
            # u = M*x - c on the replicated layout (identical fp32 ops -> same c)
            nc.vector.tensor_scalar(vrb[:], xrb[:], float(M), float(M) - 0.5, mult, amin)
            nc.vector.tensor_scalar(wrb[:], vrb[:], 0.0, big - 0.5, amax, add)
            nc.vector.tensor_scalar(wrb[:], wrb[:], big, 0.0, sub, add)
            nc.vector.tensor_tensor(vrb[:], vrb[:], wrb[:], sub)
            for ch in range(NCHUNK):
                s1 = slice(ch * NIC, (ch + 1) * NIC)
                nc.gpsimd.ap_gather(
                    out_ap=pqs[ch][:], in_ap=tb[:], idxs_ap=ibs[ch][:],
                    channels=P, num_elems=M, d=2, num_idxs=NIC,
                )
                pg = pqs[ch][:, 0::2]
                qg = pqs[ch][:, 1::2]
                nc.vector.tensor_tensor(yb[:, s1], vrb[:, s1], pg, mult)
                nc.vector.tensor_tensor(yb[:, s1], yb[:, s1], qg, add)
                nc.gpsimd.dma_start(out[:, s1], yb[:, s1])
    nc.compile()
    _PROGRAM = nc
    return nc


def _make_inmaps(points, theta, basis):
    """Per-NeuronCore input dicts. Point (j, i) of NeuronCore n is
    points[0][n*NPC + j*NI + i] (j = Q7 core, i = its stream position)."""
    tables = _grid_tables(theta, basis)           # [8, M, 2] fp16-as-uint16
    # channel ch = 16j + m holds theta (m % 8)'s pair-table; same for all NC
    tabs = np.empty((P, M * 2), dtype=np.uint16)
    for m in range(16):
        t = m % 8
        for j in range(8):
            tabs[16 * j + m] = tables[t].reshape(-1)
    x = points[0].astype(np.float32)
    in_maps = []
    for n in range(NCORES):
        sl = x[n * NPC:(n + 1) * NPC].reshape(8, NI)   # [j, i]
        # xw[16j + p, s] = x of point (j, 16s + p)  (ap_gather wrapped order)
        xw = np.empty((P, NI // 16), dtype=np.float32)
        # xr[16j + m, i] = x of point (j, i), replicated over the 16 channels
        xr = np.empty((P, NI), dtype=np.float32)
        for j in range(8):
            xw[16 * j:16 * j + 16] = sl[j].reshape(NI // 16, 16).T
            xr[16 * j:16 * j + 16] = sl[j][None, :]
        in_maps.append({"tabs": tabs, "xw": xw, "xr": xr})
    return in_maps


def _assemble(results):
    """results[n]["out"] [P, NI] -> full [8, 1, N_POINTS]."""
    out = np.empty((N_THETA, 1, N_POINTS), dtype=np.float32)
    for n in range(NCORES):
        y = results[n]["out"]
        for t in range(N_THETA):
            for j in range(8):
                out[t, 0, n * NPC + j * NI:(n * NPC) + (j + 1) * NI] = y[16 * j + t]
    return out


def kernel(points, theta, basis):
    from concourse.bass_utils import run_bass_kernel_spmd

    points = np.asarray(points)
    theta = np.asarray(theta)
    basis = np.asarray(basis)
    assert points.shape == (1, N_POINTS) and theta.shape[0] == N_THETA

    nc = _build_program()
    in_maps = _make_inmaps(points, theta, basis)
    res = run_bass_kernel_spmd(nc, in_maps, list(range(NCORES)))
    return _assemble(res.results)
